# revision 18
# baseline (speedup 1.0000x reference)
"""3x3 zero-padded window NMS (CenterNet points) on 8 trn2 NeuronCores.

points: [16, 80, 128, 128] f32 in [0,1).  out = where(p == 3x3_local_max, p, 0).

Strategy
--------
Pure data parallel over the 1280 (b,c) planes: core k owns planes
[160k, 160k+160).  Host zero-pads each plane to 130x130 so the kernel has
no edge cases.

Per-core layout: planes on SBUF partitions.  A tile covers 32 planes x
4 vertical strips (= 128 partitions), each strip 32 output rows + 2 halo
rows, full 130-col width.  All shifts are free-dim AP shifts.

Compute (per tile, all exact fp32, all on DVE -- on this toolchain the
DVE is the only engine that can run two-tensor elementwise ops; walrus'
V3 codegen rejects TensorTensor/TensorReduce on Pool and Activation):

Pairwise 3-tap max (van-Herk-style sharing, 1.5 ops/elem instead of 2):
  vertical   g[k]  = max(p[2k], p[2k+1])           17 pair rows
             Vr[2k]   = max(g[k], p[2k+2])         odd  out rows
             Vr[2k-1] = max(g[k], p[2k-1])         even out rows
  horizontal gh[k] = max(Vr[:,2k], Vr[:,2k+1])     65 pair cols
             V[:,2k]   = max(gh[k], Vr[:,2k+2])    odd  padded cols
             V[:,2k-1] = max(gh[k], Vr[:,2k-1])    even padded cols
  out = select(V - p < 2^-24, p, 0)                fused custom DVE op
Work/group: 2210+2x2080 vertical + 2080+2x2048 horizontal + 4096 select
= 16642 elems vs 20770 for the naive separable form (DVE is 1 elem/cycle
fp32 regardless of op, so fewer elements = proportionally faster).

Select offload: the otherwise-idle PE + ACT engines take PE_ROWS of the
32 select rows via an exact matmul trick (HW-validated bit-exact):
  psum  = 2^25*I @ p        (fp32 matmul, pow2 weights -> exact)
  psum -= 2^25*I @ V        (exact cancellation: p,V mult. of 2^-24)
  psum += I @ p             (0 + p = p exact, or stays < 0)
  out   = ACT relu(psum)    = p iff V <= p else 0
fp32 matmuls run 4 cycles/row (exact IEEE products), so PE costs
5 ns/select-elem vs DVE 1.04 -- but it runs in parallel, off the DVE
critical path.  The DVE keeps the remaining 32-PE_ROWS rows.

Inputs are multiples of 2^-23 (jax.random.uniform), so V - p is exact in
fp32: 0 iff p is the window max, else >= 2^-23 -> the select is bit-exact.

Perf notes (HW-measured):
 - fp32 two-tensor elementwise ops run at 1 elem/cycle on the DVE (the
   2x/4x DVE perf modes only cover one-tensor ops).
 - The DVE stalls ~op-duration when an op consumes the *immediately*
   previous op's output.  The per-group op order software-pipelines the
   vertical stage of group g+1 between the horizontal/select ops of
   group g: [gh(g), gv(g+1), Va(g), Vra(g+1), Vb(g), Vrb(g+1), Se(g),
   So(g)] -- every producer->consumer pair is >= 2 instructions apart.
 - DMA APs keep the 32-plane dim outermost (HWDGE ring fan-out keys on it;
   3x bandwidth vs strip-outermost).
 - Loads prefetch 3 groups ahead and are emitted before stores so the
   in-order SP queue never holds a needed load behind a store's wait.
"""

import numpy as np

import concourse.bass as bass
import concourse.bacc as bacc
import concourse.mybir as mybir
import concourse.dve_ops as dve_ops
from concourse.dve_spec import Spec, Src0, Src1, C0, Zero, select, lower
from concourse.dve_uop import DveOpSpec
from concourse.tile import TileContext
from concourse.bass_utils import run_bass_kernel_spmd


def _register_nms_select():
    """Fused NMS select as a custom DVE op:
        out = Src0 if (Src1 - Src0) < s0 else 0      (Src0=p, Src1=V=3x3max)
    With s0 = 2^-24: V - p is exact in fp32 (inputs are multiples of 2^-23),
    zero iff p is the window max, else >= 2^-23 -> bit-exact select in ONE
    DVE pass, replacing sub + scalar_tensor_tensor + ACT relu."""
    name = "NMS_SELECT_ANT"
    if name in dve_ops._SUB_OPCODE_FOR_NAME:
        return next(o for o in dve_ops.OPS if o.name == name)
    spec = Spec(
        body=select(Src1 - Src0 < C0, Src0, Zero),
        reference=lambda in0, in1, s0, s1, imm2: np.where(
            (in1.astype(np.float32).reshape(in0.shape) - in0) < s0, in0, 0.0
        ).astype(np.float32),
    )
    # Self-pin the uops sha (the pin exists to catch lowering drift of
    # in-repo ops; for a runtime-registered op we pin to what we lower now).
    shas = {}
    for ver in ("v3", "v4"):
        try:
            s = DveOpSpec(name=name, opcode=0, uops=lower(spec, ver=ver),
                          rd1_en=True)
            shas[ver] = s.sha(ver)
        except Exception:
            pass
    op = dve_ops.DveOp(name, spec, subdim=False, uops_sha=shas)
    row = max(dve_ops._SUB_OPCODE_FOR_NAME.values()) + 1
    assert row < 0x20
    dve_ops.OPS.append(op)
    dve_ops.CUSTOM_DVE_SPECS[name] = spec
    dve_ops._SUB_OPCODE_FOR_NAME[name] = row
    return op


NMS_SELECT = _register_nms_select()
EPS_SEL = float(2.0 ** -24)

B, C, H, W = 16, 80, 128, 128
NCORES = 8
PLANES = B * C            # 1280
PPC = PLANES // NCORES    # 160 planes per core
GP = 32                   # planes per tile-group
NST = 4                   # vertical strips per plane
SR = H // NST             # 32 output rows per strip
NG = PPC // GP            # 5 groups per core
HP = H + 2                # 130 padded
WP = W + 2                # 130 padded
F32 = mybir.dt.float32
PE_ROWS = 20              # select rows on PE (5 PSUM chunks x 4 rows)
CHUNK_R = 4               # rows per PSUM chunk (4*128 = 512 = max moving)
NCH = PE_ROWS // CHUNK_R  # PSUM chunks per group
SEL_C = float(2.0 ** 25)  # select scale (>= 2^24 so any gap kills relu)

_CACHE = {}
LAST_RESULT = None        # BassKernelResults of the most recent run


def _build_program(repeat: int = 1, mode: str = "full"):
    # Bacc (not raw Bass): its compile pipeline runs generate_event_semaphores,
    # which splits multi-wait instructions to satisfy the TRN2 1-wait-per-
    # instruction ISA constraint.
    nc = bacc.Bacc()
    x = nc.dram_tensor("x", [PPC, HP, WP], F32, kind="ExternalInput")
    w = nc.dram_tensor("w", [3, 128, 128], F32, kind="ExternalInput")
    y = nc.dram_tensor("y", [PPC, H, W], F32, kind="ExternalOutput")
    xap = x[:]
    yap = y[:]

    glist = [g for _ in range(repeat) for g in range(NG)]
    tins = {}
    verts = {}
    PF = 3  # load prefetch distance (tin bufs = PF + 1)

    def _emit_load(gi):
        # DRAM side iterates (plane, strip, row, col) so that partition
        # p = plane*NST + strip; strips overlap by 2 rows.  Plane (count 32)
        # outermost: the HWDGE queue fan-out keys on the outer dim, and 32
        # spreads across all rings (3x DMA BW vs strip-outermost).
        t = pool.tile([128, SR + 2, WP], F32, tag="tin", bufs=PF + 1, name="tin")
        src = bass.AP(
            xap.tensor,
            glist[gi] * GP * HP * WP,
            [[HP * WP, GP], [SR * WP, NST], [1, (SR + 2) * WP]],
        )
        if mode != "nodma":
            nc.sync.dma_start(out=t[:], in_=src)
        else:
            # ACT-engine memzero: keeps the nodma diagnostic from adding
            # work to Pool/DVE, which now both carry real compute.
            nc.scalar.memzero(t[:])
        tins[gi] = t

    def _emit_gv(gj):
        """Vertical pair stage of group gj: g[k] = max(tin[2k], tin[2k+1])."""
        tin = tins[gj]
        gv = pool.tile([128, 17, WP], F32, tag="gv", bufs=2)
        nc.vector.tensor_max(gv[:], tin[:, 0:34:2, :], tin[:, 1:34:2, :])
        verts[gj] = (gv, None)

    def _emit_vra(gj):
        """Odd output rows r=2k+1: Vr[2k] = max(g[k], tin[2k+2])."""
        tin = tins[gj]
        gv, _ = verts[gj]
        Vr = pool.tile([128, SR, WP], F32, tag="Vr", bufs=2)
        nc.vector.tensor_max(
            Vr[:, 0:SR:2, :], gv[:, 0:16, :], tin[:, 2:34:2, :]
        )
        verts[gj] = (gv, Vr)

    def _emit_vrb(gj):
        """Even output rows r=2k: Vr[2k-1] = max(g[k], tin[2k-1])."""
        tin = tins[gj]
        gv, Vr = verts[gj]
        nc.vector.tensor_max(
            Vr[:, 1:SR:2, :], gv[:, 1:17, :], tin[:, 1:33:2, :]
        )

    with TileContext(nc) as tc:
        with tc.tile_pool(name="pool", bufs=1) as pool, \
             tc.tile_pool(name="ppool", space="PSUM", bufs=1) as ppool:
            # Select weights: [128part(K), 3, 128(M)] = diag(2^25, -2^25, 1),
            # loaded once; glist repeats reuse the same SBUF tile.
            tw = pool.tile([128, 3, 128], F32, tag="tw", bufs=1)
            if mode != "nodma":
                wsrc = bass.AP(w[:].tensor, 0, [[128, 128], [128 * 128, 3], [1, 128]])
                nc.sync.dma_start(out=tw[:], in_=wsrc)
            else:
                nc.scalar.memzero(tw[:])
            for gi, g in enumerate(glist):
                # Loads run PF groups ahead of compute, and are emitted
                # before this group's store so the in-order SP queue can
                # never hold a needed load behind a store's wait.
                if gi == 0:
                    for j in range(min(PF, len(glist))):
                        _emit_load(j)
                if gi + PF < len(glist):
                    _emit_load(gi + PF)
                tin = tins[gi]
                if mode == "dmaonly":
                    dst = bass.AP(
                        yap.tensor,
                        g * GP * H * W,
                        [[H * W, GP], [SR * W, NST], [1, SR * W]],
                    )
                    tin_flat = bass.AP(
                        tin.tensor, tin.offset, [[(SR + 2) * WP, 128], [1, SR * W]]
                    )
                    nc.sync.dma_start(out=dst, in_=tin_flat)
                    tins.pop(gi)
                    continue

                # Software-pipelined order: the vertical stage of group
                # gi+1 is interleaved between the horizontal/select ops of
                # group gi so every producer->consumer pair is >= 2 DVE
                # instructions apart (distance-1 chains stall ~op-duration).
                if gi == 0:
                    _emit_gv(0)
                    _emit_vra(0)
                    _emit_vrb(0)
                _, Vr = verts[gi]
                gh = pool.tile([128, SR, 65], F32, tag="gh", bufs=1)
                V = pool.tile([128, SR, W], F32, tag="V", bufs=1)
                tout = pool.tile([128, SR, W], F32, tag="tout", bufs=3)
                pss = [
                    ppool.tile([128, CHUNK_R, W], F32, tag=f"ps{c}", bufs=1,
                               name=f"ps{c}")
                    for c in range(NCH)
                ]

                # PE pass 1 (needs only tin): psum[c] = 2^25 * p_chunk
                for c in range(NCH):
                    r0 = 1 + c * CHUNK_R
                    nc.tensor.matmul(
                        out=pss[c][:], lhsT=tw[:, 0, :],
                        rhs=tin[:, r0:r0 + CHUNK_R, 1:WP - 1],
                        start=True, stop=False, skip_group_check=True,
                    )

                # gh[k] = max(Vr[:,2k], Vr[:,2k+1]), k=0..64
                nc.vector.tensor_max(
                    gh[:], Vr[:, :, 0:WP:2], Vr[:, :, 1:WP:2]
                )
                if gi + 1 < len(glist):
                    _emit_gv(gi + 1)
                # odd padded cols q=2k+1 -> V[:,2k] = max(gh[k], Vr[:,2k+2])
                nc.vector.tensor_max(
                    V[:, :, 0:W:2], gh[:, :, 0:64], Vr[:, :, 2:WP:2]
                )
                if gi + 1 < len(glist):
                    _emit_vra(gi + 1)
                # even padded cols q=2k -> V[:,2k-1] = max(gh[k], Vr[:,2k-1])
                nc.vector.tensor_max(
                    V[:, :, 1:W:2], gh[:, :, 1:65], Vr[:, :, 1:WP - 1:2]
                )
                if gi + 1 < len(glist):
                    _emit_vrb(gi + 1)
                # DVE select on the last SR-PE_ROWS rows, split by output
                # column parity (Se reads only Va's half of V, So only
                # Vb's -- keeps DVE producer->consumer distances >= 2)
                nc.vector._custom_dve(
                    NMS_SELECT,
                    out=tout[:, PE_ROWS:SR, 0:W:2],
                    in0=tin[:, 1 + PE_ROWS:33, 1:WP - 1:2],
                    in1=V[:, PE_ROWS:SR, 0:W:2],
                    s0=EPS_SEL,
                )
                nc.vector._custom_dve(
                    NMS_SELECT,
                    out=tout[:, PE_ROWS:SR, 1:W:2],
                    in0=tin[:, 1 + PE_ROWS:33, 2:WP:2],
                    in1=V[:, PE_ROWS:SR, 1:W:2],
                    s0=EPS_SEL,
                )

                # PE passes 2+3 (need V): psum[c] += -2^25*V_chunk + p_chunk
                for c in range(NCH):
                    r0 = c * CHUNK_R
                    nc.tensor.matmul(
                        out=pss[c][:], lhsT=tw[:, 1, :],
                        rhs=V[:, r0:r0 + CHUNK_R, :],
                        start=False, stop=False, skip_group_check=True,
                    )
                for c in range(NCH):
                    r0 = 1 + c * CHUNK_R
                    nc.tensor.matmul(
                        out=pss[c][:], lhsT=tw[:, 2, :],
                        rhs=tin[:, r0:r0 + CHUNK_R, 1:WP - 1],
                        start=False, stop=True, skip_group_check=True,
                    )
                # ACT drains each chunk: tout rows = relu(psum)
                for c in range(NCH):
                    r0 = c * CHUNK_R
                    nc.scalar.activation(
                        tout[:, r0:r0 + CHUNK_R, :], pss[c][:],
                        mybir.ActivationFunctionType.Relu,
                    )
                tins.pop(gi)
                verts.pop(gi)

                if mode != "nodma":
                    dst = bass.AP(
                        yap.tensor,
                        g * GP * H * W,
                        [[H * W, GP], [SR * W, NST], [1, SR * W]],
                    )
                    # Stores go out the idle Pool engine's SWDGE queue so
                    # the in-order SP queue carries only loads: a store
                    # waiting on compute can never delay a prefetch load.
                    nc.gpsimd.dma_start(out=dst, in_=tout[:])
    nc.finalize()
    return nc


def get_nc(repeat: int = 1, mode: str = "full"):
    key = f"nc{repeat}_{mode}"
    if key not in _CACHE:
        _CACHE[key] = _build_program(repeat, mode)
    return _CACHE[key]


def pad_input(points: np.ndarray) -> np.ndarray:
    pts = np.ascontiguousarray(points, dtype=np.float32).reshape(PLANES, H, W)
    xpad = np.zeros((PLANES, HP, WP), np.float32)
    xpad[:, 1:H + 1, 1:W + 1] = pts
    return xpad


def select_weights() -> np.ndarray:
    """PE select weights: diag(2^25), diag(-2^25), diag(1) as [3,128,128]."""
    wt = np.zeros((3, 128, 128), np.float32)
    wt[0] = np.eye(128, dtype=np.float32) * SEL_C
    wt[1] = np.eye(128, dtype=np.float32) * -SEL_C
    wt[2] = np.eye(128, dtype=np.float32)
    return wt


def kernel(**inputs) -> np.ndarray:
    global LAST_RESULT
    import os

    # The axon NTFF profile hook is absent in this environment; force the
    # non-tracing execute path even if BASS_TRACE is set externally.
    os.environ["BASS_NEVER_TRACE"] = "1"
    xpad = pad_input(inputs["points"])
    wt = select_weights()
    nc = get_nc()
    in_maps = [
        {"x": xpad[k * PPC:(k + 1) * PPC], "w": wt} for k in range(NCORES)
    ]
    res = run_bass_kernel_spmd(nc, in_maps, list(range(NCORES)))
    LAST_RESULT = res
    full = np.empty((PLANES, H, W), np.float32)
    for k in range(NCORES):
        full[k * PPC:(k + 1) * PPC] = res.results[k]["y"]
    return full.reshape(B, C, H, W)



# revision 19
# speedup vs baseline: 1.1992x; 1.1992x over previous
"""3x3 zero-padded window NMS (CenterNet points) on 8 trn2 NeuronCores.

points: [16, 80, 128, 128] f32 in [0,1).  out = where(p == 3x3_local_max, p, 0).

Strategy
--------
Pure data parallel over the 1280 (b,c) planes: core k owns planes
[160k, 160k+160).  Host zero-pads each plane to 130x130 so the kernel has
no edge cases.

Per-core layout: planes on SBUF partitions.  A tile covers 32 planes x
4 vertical strips (= 128 partitions), each strip 32 output rows + 2 halo
rows, full 130-col width.  All shifts are free-dim AP shifts.

Compute (per tile, all exact fp32, all on DVE -- on this toolchain the
DVE is the only engine that can run two-tensor elementwise ops; walrus'
V3 codegen rejects TensorTensor/TensorReduce on Pool and Activation):

Pairwise 3-tap max (van-Herk-style sharing, 1.5 ops/elem instead of 2):
  vertical   g[k]  = max(p[2k], p[2k+1])           17 pair rows
             Vr[2k]   = max(g[k], p[2k+2])         odd  out rows
             Vr[2k-1] = max(g[k], p[2k-1])         even out rows
  horizontal gh[k] = max(Vr[:,2k], Vr[:,2k+1])     65 pair cols
             V[:,2k]   = max(gh[k], Vr[:,2k+2])    odd  padded cols
             V[:,2k-1] = max(gh[k], Vr[:,2k-1])    even padded cols
  out = select(V - p < 2^-24, p, 0)                fused custom DVE op
Work/group: 2210+2x2080 vertical + 2080+2x2048 horizontal + 4096 select
= 16642 elems vs 20770 for the naive separable form (DVE is 1 elem/cycle
fp32 regardless of op, so fewer elements = proportionally faster).

Select offload: the otherwise-idle PE + ACT engines take PE_ROWS of the
32 select rows via an exact matmul trick (HW-validated bit-exact):
  psum  = 2^25*I @ p        (fp32 matmul, pow2 weights -> exact)
  psum -= 2^25*I @ V        (exact cancellation: p,V mult. of 2^-24)
  psum += I @ p             (0 + p = p exact, or stays < 0)
  out   = ACT relu(psum)    = p iff V <= p else 0
fp32 matmuls run 4 cycles/row (exact IEEE products), so PE costs
5 ns/select-elem vs DVE 1.04 -- but it runs in parallel, off the DVE
critical path.  The DVE keeps the remaining 32-PE_ROWS rows.

Inputs are multiples of 2^-23 (jax.random.uniform), so V - p is exact in
fp32: 0 iff p is the window max, else >= 2^-23 -> the select is bit-exact.

Perf notes (HW-measured):
 - fp32 two-tensor elementwise ops run at 1 elem/cycle on the DVE (the
   2x/4x DVE perf modes only cover one-tensor ops).
 - The DVE stalls ~op-duration when an op consumes the *immediately*
   previous op's output.  The per-group op order software-pipelines the
   vertical stage of group g+1 between the horizontal/select ops of
   group g: [gh(g), gv(g+1), Va(g), Vra(g+1), Vb(g), Vrb(g+1), Se(g),
   So(g)] -- every producer->consumer pair is >= 2 instructions apart.
 - DMA APs keep the 32-plane dim outermost (HWDGE ring fan-out keys on it;
   3x bandwidth vs strip-outermost).
 - Loads prefetch 3 groups ahead and are emitted before stores so the
   in-order SP queue never holds a needed load behind a store's wait.
"""

import numpy as np

import concourse.bass as bass
import concourse.bacc as bacc
import concourse.mybir as mybir
import concourse.dve_ops as dve_ops
from concourse.dve_spec import Spec, Src0, Src1, C0, Zero, select, lower
from concourse.dve_uop import DveOpSpec
from concourse.tile import TileContext
from concourse.bass_utils import run_bass_kernel_spmd


def _register_nms_select():
    """Fused NMS select as a custom DVE op:
        out = Src0 if (Src1 - Src0) < s0 else 0      (Src0=p, Src1=V=3x3max)
    With s0 = 2^-24: V - p is exact in fp32 (inputs are multiples of 2^-23),
    zero iff p is the window max, else >= 2^-23 -> bit-exact select in ONE
    DVE pass, replacing sub + scalar_tensor_tensor + ACT relu."""
    name = "NMS_SELECT_ANT"
    if name in dve_ops._SUB_OPCODE_FOR_NAME:
        return next(o for o in dve_ops.OPS if o.name == name)
    spec = Spec(
        body=select(Src1 - Src0 < C0, Src0, Zero),
        reference=lambda in0, in1, s0, s1, imm2: np.where(
            (in1.astype(np.float32).reshape(in0.shape) - in0) < s0, in0, 0.0
        ).astype(np.float32),
    )
    # Self-pin the uops sha (the pin exists to catch lowering drift of
    # in-repo ops; for a runtime-registered op we pin to what we lower now).
    shas = {}
    for ver in ("v3", "v4"):
        try:
            s = DveOpSpec(name=name, opcode=0, uops=lower(spec, ver=ver),
                          rd1_en=True)
            shas[ver] = s.sha(ver)
        except Exception:
            pass
    op = dve_ops.DveOp(name, spec, subdim=False, uops_sha=shas)
    row = max(dve_ops._SUB_OPCODE_FOR_NAME.values()) + 1
    assert row < 0x20
    dve_ops.OPS.append(op)
    dve_ops.CUSTOM_DVE_SPECS[name] = spec
    dve_ops._SUB_OPCODE_FOR_NAME[name] = row
    return op


NMS_SELECT = _register_nms_select()
EPS_SEL = float(2.0 ** -24)

B, C, H, W = 16, 80, 128, 128
NCORES = 8
PLANES = B * C            # 1280
PPC = PLANES // NCORES    # 160 planes per core
GP = 32                   # planes per tile-group
NST = 4                   # vertical strips per plane
SR = H // NST             # 32 output rows per strip
NG = PPC // GP            # 5 groups per core
HP = H + 2                # 130 padded
WP = W + 2                # 130 padded
F32 = mybir.dt.float32
PE_ROWS = 20              # select rows on PE (5 PSUM chunks x 4 rows)
CHUNK_R = 4               # rows per PSUM chunk (4*128 = 512 = max moving)
NCH = PE_ROWS // CHUNK_R  # PSUM chunks per group
SEL_C = float(2.0 ** 25)  # select scale (>= 2^24 so any gap kills relu)

_CACHE = {}
LAST_RESULT = None        # BassKernelResults of the most recent run


def _build_program(repeat: int = 1, mode: str = "full"):
    # Bacc (not raw Bass): its compile pipeline runs generate_event_semaphores,
    # which splits multi-wait instructions to satisfy the TRN2 1-wait-per-
    # instruction ISA constraint.
    nc = bacc.Bacc()
    x = nc.dram_tensor("x", [PPC, HP, WP], F32, kind="ExternalInput")
    w = nc.dram_tensor("w", [3, 128, 128], F32, kind="ExternalInput")
    y = nc.dram_tensor("y", [PPC, H, W], F32, kind="ExternalOutput")
    xap = x[:]
    yap = y[:]

    glist = [g for _ in range(repeat) for g in range(NG)]
    tins = {}
    verts = {}
    PF = 3  # load prefetch distance (tin bufs = PF + 1)

    def _emit_load(gi):
        # DRAM side iterates (plane, strip, row, col) so that partition
        # p = plane*NST + strip; strips overlap by 2 rows.  Plane (count 32)
        # outermost: the HWDGE queue fan-out keys on the outer dim, and 32
        # spreads across all rings (3x DMA BW vs strip-outermost).
        t = pool.tile([128, SR + 2, WP], F32, tag="tin", bufs=PF + 1, name="tin")
        src = bass.AP(
            xap.tensor,
            glist[gi] * GP * HP * WP,
            [[HP * WP, GP], [SR * WP, NST], [1, (SR + 2) * WP]],
        )
        if mode != "nodma":
            nc.sync.dma_start(out=t[:], in_=src)
        else:
            # ACT-engine memzero: keeps the nodma diagnostic from adding
            # work to Pool/DVE, which now both carry real compute.
            nc.scalar.memzero(t[:])
        tins[gi] = t

    def _emit_gv(gj):
        """Vertical pair stage of group gj: g[k] = max(tin[2k], tin[2k+1])."""
        tin = tins[gj]
        gv = pool.tile([128, 17, WP], F32, tag="gv", bufs=2)
        nc.vector.tensor_max(gv[:], tin[:, 0:34:2, :], tin[:, 1:34:2, :])
        verts[gj] = (gv, None)

    def _emit_vra(gj):
        """Odd output rows r=2k+1: Vr[2k] = max(g[k], tin[2k+2])."""
        tin = tins[gj]
        gv, _ = verts[gj]
        Vr = pool.tile([128, SR, WP], F32, tag="Vr", bufs=2)
        nc.vector.tensor_max(
            Vr[:, 0:SR:2, :], gv[:, 0:16, :], tin[:, 2:34:2, :]
        )
        verts[gj] = (gv, Vr)

    def _emit_vrb(gj):
        """Even output rows r=2k: Vr[2k-1] = max(g[k], tin[2k-1])."""
        tin = tins[gj]
        gv, Vr = verts[gj]
        nc.vector.tensor_max(
            Vr[:, 1:SR:2, :], gv[:, 1:17, :], tin[:, 1:33:2, :]
        )

    with TileContext(nc) as tc:
        with tc.tile_pool(name="pool", bufs=1) as pool, \
             tc.tile_pool(name="ppool", space="PSUM", bufs=1) as ppool:
            # Select weights: [128part(K), 3, 128(M)] = diag(2^25, -2^25, 1),
            # loaded once; glist repeats reuse the same SBUF tile.
            tw = pool.tile([128, 3, 128], F32, tag="tw", bufs=1)
            if mode != "nodma":
                wsrc = bass.AP(w[:].tensor, 0, [[128, 128], [128 * 128, 3], [1, 128]])
                nc.sync.dma_start(out=tw[:], in_=wsrc)
            else:
                nc.scalar.memzero(tw[:])
            for gi, g in enumerate(glist):
                # Loads run PF groups ahead of compute, and are emitted
                # before this group's store so the in-order SP queue can
                # never hold a needed load behind a store's wait.
                if gi == 0:
                    for j in range(min(PF, len(glist))):
                        _emit_load(j)
                if gi + PF < len(glist):
                    _emit_load(gi + PF)
                tin = tins[gi]
                if mode == "dmaonly":
                    dst = bass.AP(
                        yap.tensor,
                        g * GP * H * W,
                        [[H * W, GP], [SR * W, NST], [1, SR * W]],
                    )
                    tin_flat = bass.AP(
                        tin.tensor, tin.offset, [[(SR + 2) * WP, 128], [1, SR * W]]
                    )
                    nc.sync.dma_start(out=dst, in_=tin_flat)
                    tins.pop(gi)
                    continue

                # Software-pipelined order: the vertical stage of group
                # gi+1 is interleaved between the horizontal/select ops of
                # group gi so every producer->consumer pair is >= 2 DVE
                # instructions apart (distance-1 chains stall ~op-duration).
                if gi == 0:
                    _emit_gv(0)
                    _emit_vra(0)
                    _emit_vrb(0)
                _, Vr = verts[gi]
                gh = pool.tile([128, SR, 65], F32, tag="gh", bufs=1)
                V = pool.tile([128, SR, W], F32, tag="V", bufs=1)
                tout = pool.tile([128, SR, W], F32, tag="tout", bufs=3)
                pss = [
                    ppool.tile([128, CHUNK_R, W], F32, tag=f"ps{c}", bufs=1,
                               name=f"ps{c}")
                    for c in range(NCH)
                ]

                # PE pass 1 (needs only tin): psum[c] = 2^25 * p_chunk
                for c in range(NCH):
                    r0 = 1 + c * CHUNK_R
                    nc.tensor.matmul(
                        out=pss[c][:], lhsT=tw[:, 0, :],
                        rhs=tin[:, r0:r0 + CHUNK_R, 1:WP - 1],
                        start=True, stop=False, skip_group_check=True,
                    )

                # gh[k] = max(Vr[:,2k], Vr[:,2k+1]), k=0..64
                nc.vector.tensor_max(
                    gh[:], Vr[:, :, 0:WP:2], Vr[:, :, 1:WP:2]
                )
                if gi + 1 < len(glist):
                    _emit_gv(gi + 1)
                # odd padded cols q=2k+1 -> V[:,2k] = max(gh[k], Vr[:,2k+2])
                nc.vector.tensor_max(
                    V[:, :, 0:W:2], gh[:, :, 0:64], Vr[:, :, 2:WP:2]
                )
                if gi + 1 < len(glist):
                    _emit_vra(gi + 1)
                # even padded cols q=2k -> V[:,2k-1] = max(gh[k], Vr[:,2k-1])
                nc.vector.tensor_max(
                    V[:, :, 1:W:2], gh[:, :, 1:65], Vr[:, :, 1:WP - 1:2]
                )
                if gi + 1 < len(glist):
                    _emit_vrb(gi + 1)
                # DVE select on the last SR-PE_ROWS rows, split by output
                # column parity (Se reads only Va's half of V, So only
                # Vb's -- keeps DVE producer->consumer distances >= 2)
                nc.vector._custom_dve(
                    NMS_SELECT,
                    out=tout[:, PE_ROWS:SR, 0:W:2],
                    in0=tin[:, 1 + PE_ROWS:33, 1:WP - 1:2],
                    in1=V[:, PE_ROWS:SR, 0:W:2],
                    s0=EPS_SEL,
                )
                nc.vector._custom_dve(
                    NMS_SELECT,
                    out=tout[:, PE_ROWS:SR, 1:W:2],
                    in0=tin[:, 1 + PE_ROWS:33, 2:WP:2],
                    in1=V[:, PE_ROWS:SR, 1:W:2],
                    s0=EPS_SEL,
                )

                # PE passes 2+3 (need V): psum[c] += -2^25*V_chunk + p_chunk
                for c in range(NCH):
                    r0 = c * CHUNK_R
                    nc.tensor.matmul(
                        out=pss[c][:], lhsT=tw[:, 1, :],
                        rhs=V[:, r0:r0 + CHUNK_R, :],
                        start=False, stop=False, skip_group_check=True,
                    )
                for c in range(NCH):
                    r0 = 1 + c * CHUNK_R
                    nc.tensor.matmul(
                        out=pss[c][:], lhsT=tw[:, 2, :],
                        rhs=tin[:, r0:r0 + CHUNK_R, 1:WP - 1],
                        start=False, stop=True, skip_group_check=True,
                    )
                # ACT drains each chunk: tout rows = relu(psum)
                for c in range(NCH):
                    r0 = c * CHUNK_R
                    nc.scalar.activation(
                        tout[:, r0:r0 + CHUNK_R, :], pss[c][:],
                        mybir.ActivationFunctionType.Relu,
                    )
                tins.pop(gi)
                verts.pop(gi)

                if mode != "nodma":
                    dst = bass.AP(
                        yap.tensor,
                        g * GP * H * W,
                        [[H * W, GP], [SR * W, NST], [1, SR * W]],
                    )
                    nc.sync.dma_start(out=dst, in_=tout[:])
    nc.finalize()
    return nc


def get_nc(repeat: int = 1, mode: str = "full"):
    key = f"nc{repeat}_{mode}"
    if key not in _CACHE:
        _CACHE[key] = _build_program(repeat, mode)
    return _CACHE[key]


def pad_input(points: np.ndarray) -> np.ndarray:
    pts = np.ascontiguousarray(points, dtype=np.float32).reshape(PLANES, H, W)
    xpad = np.zeros((PLANES, HP, WP), np.float32)
    xpad[:, 1:H + 1, 1:W + 1] = pts
    return xpad


def select_weights() -> np.ndarray:
    """PE select weights: diag(2^25), diag(-2^25), diag(1) as [3,128,128]."""
    wt = np.zeros((3, 128, 128), np.float32)
    wt[0] = np.eye(128, dtype=np.float32) * SEL_C
    wt[1] = np.eye(128, dtype=np.float32) * -SEL_C
    wt[2] = np.eye(128, dtype=np.float32)
    return wt


def kernel(**inputs) -> np.ndarray:
    global LAST_RESULT
    import os

    # The axon NTFF profile hook is absent in this environment; force the
    # non-tracing execute path even if BASS_TRACE is set externally.
    os.environ["BASS_NEVER_TRACE"] = "1"
    xpad = pad_input(inputs["points"])
    wt = select_weights()
    nc = get_nc()
    in_maps = [
        {"x": xpad[k * PPC:(k + 1) * PPC], "w": wt} for k in range(NCORES)
    ]
    res = run_bass_kernel_spmd(nc, in_maps, list(range(NCORES)))
    LAST_RESULT = res
    full = np.empty((PLANES, H, W), np.float32)
    for k in range(NCORES):
        full[k * PPC:(k + 1) * PPC] = res.results[k]["y"]
    return full.reshape(B, C, H, W)

